# revision 42
# baseline (speedup 1.0000x reference)
"""Trainium2 Bass kernel for MemoryEfficientAttention (B=4, S=2048, D=1024, H=16).

Sharding: 8 cores = 4 batches x 2 head-groups (8 heads each).
Each core computes qkv projection for its head group, attention, and a
row-parallel partial of the output projection. Host sums the two partials
per batch and folds the (zero) biases.

Schedule: the v-projection overlaps the input DMA (dd-outer accumulation);
the tail of the v-projection, the q/k projection, and the output projection
all run as background PE tasks pumped into the attention loop's slack, so
the ScalarE exp stream (the pacing engine) starts early and never waits at
the end. Softmax normalization is fused per (pair, q-chunk) drain using a
GPSIMD partition-broadcast of the reciprocal denominators (no DRAM bounce).
"""

import sys
from collections import deque
from contextlib import ExitStack

if "/opt/trn_rl_repo" not in sys.path:
    sys.path.insert(0, "/opt/trn_rl_repo")

import numpy as np

import concourse.bass as bass
import concourse.mybir as mybir
import concourse.tile as tile
from concourse import bacc

F32 = mybir.dt.float32
EXP = mybir.ActivationFunctionType.Exp

S = 2048          # sequence length
D = 1024          # model dim
HG = 8            # heads per core (group)
DH = 64           # head dim
DK = HG * DH      # 512, per-core attention dim
NKT = S // 128    # 16 key tiles
NQT = S // 128    # 16 query/token tiles
NDT = D // 128    # 8 d-tiles
Q4 = 512          # query chunk


def build_program():
    """Build the SPMD Bass/Tile program (same program on all 8 cores)."""
    nc = bacc.Bacc("TRN2")
    BF16 = mybir.dt.bfloat16

    xT = nc.dram_tensor("xT", [D, S], BF16, kind="ExternalInput").ap()
    # wqk: 8 column-tiles (q cols 0-511 scaled by 1/8, then k cols), tiled
    # [ct, dt, 128, 128] so each DMA is one contiguous 64KB block.
    wqk = nc.dram_tensor("wqk", [8, NDT, 128, 128], BF16, kind="ExternalInput").ap()
    # wv: [dt, 128, 512] row-blocks of the v projection.
    wv = nc.dram_tensor("wv", [NDT, 128, DK], BF16, kind="ExternalInput").ap()
    # bqk: q bias (pre-scaled) then k bias, laid out [128, 8] partition-major.
    bqk = nc.dram_tensor("bqk", [D], F32, kind="ExternalInput").ap()
    wout = nc.dram_tensor("wout", [DK, D], BF16, kind="ExternalInput").ap()
    # bf16 partial output: halves the writeback DMA; host sums in fp32
    out = nc.dram_tensor("out", [S, D], BF16, kind="ExternalOutput").ap()


    with tile.TileContext(nc) as tc, ExitStack() as ctx:
        persist = ctx.enter_context(tc.tile_pool(name="persist", bufs=1))
        # qT/kT: transposed projections, (dh x tokens) per head; head h lives
        # in tile column h//2 at partitions (h%2)*64 .. +64.
        qT = persist.tile([128, 4, S], BF16, tag="qT")
        kT = persist.tile([128, 4, S], BF16, tag="kT")
        bias_sb = persist.tile([128, 8], F32, tag="bias_sb")
        wout_sb = persist.tile([128, 4, D], BF16, tag="wout_sb")
        attT = persist.tile([128, 4, S], BF16, tag="attT")
        xT_sb = persist.tile([128, NDT, S], BF16, tag="xT_sb")
        wqk_sb = persist.tile([128, 8, NDT, 128], BF16, tag="wqk_sb")
        wv_sb = persist.tile([128, NDT, DK], BF16, tag="wv_sb")
        # v in natural layout (bf16), augmented with a ones column per head:
        # v_sb[:, kt, h, 0:64] = v tokens kt*128.., head h; [..., 64] = 1.0
        v_sb = persist.tile([128, NKT, HG, DH + 1], BF16, tag="v_sb")

        # small rotating buffers for the per-chunk softmax normalization
        nrm = ctx.enter_context(tc.tile_pool(name="nrm", bufs=3))

        nc.vector.memset(v_sb[:, :, :, DH : DH + 1].bitcast(mybir.dt.uint16), 0x3F80)

        # ---- input DMAs, split across queues; wv first so the v-projection
        # can start as soon as x d-tile 0 lands.
        for dd in range(NDT):
            nc.sync.dma_start(out=xT_sb[:, dd, :], in_=xT[dd * 128 : (dd + 1) * 128, :])
        for j in range(4):
            nc.sync.dma_start(out=wout_sb[:, j, :], in_=wout[j * 128 : (j + 1) * 128, :])
        for dd in range(NDT):
            nc.scalar.dma_start(out=wv_sb[:, dd, :], in_=wv[dd])
        nc.scalar.dma_start(out=bias_sb, in_=bqk.rearrange("(c p) -> p c", p=128))
        for ct in (0, 4, 1, 5, 2, 6, 3, 7):
            nc.scalar.dma_start(
                out=wqk_sb[:, ct, :, :], in_=wqk[ct].rearrange("d p k -> p d k")
            )

        # ---- head phase 1: v-projection for token tiles 0-9, dd-outer so
        # the PSUM accumulation starts after the first x d-tile arrives.
        with ExitStack() as p1b:
            vps_pool = p1b.enter_context(tc.tile_pool(name="vps", bufs=8, space="PSUM"))
            for tset in (range(0, 8), range(8, 10)):
                vt = {}
                for t in tset:
                    vt[t] = vps_pool.tile([128, DK], F32, tag="vps", name=f"vps{t}")
                for dd in range(NDT):
                    for t in tset:
                        nc.tensor.matmul(
                            vt[t],
                            xT_sb[:, dd, t * 128 : (t + 1) * 128],
                            wv_sb[:, dd, :],
                            start=(dd == 0),
                            stop=(dd == NDT - 1),
                        )
                for t in tset:
                    nc.vector.tensor_copy(
                        out=v_sb[:, t, :, 0:DH],
                        in_=vt[t].rearrange("p (h e) -> p h e", h=HG),
                    )

        # ---- background-task PSUM pool: q/k projection accumulators, the
        # remaining v-projection tiles, and later the out-projection chunks
        # all use the same [128, 512] fp32 shape.
        bgpool = ctx.enter_context(tc.tile_pool(name="bgp", bufs=2, space="PSUM"))

        # q/k projection stepper: one N=512 matmul per step; a step sequence
        # covers c-tiles in pair order so pair j+1's tiles finish during pair
        # j's attention.
        qk_state = {"ps": None}
        qk_steps = [
            (ct, chunk, dd)
            for ct in (0, 4, 1, 5, 2, 6, 3, 7)
            for chunk in range(4)
            for dd in range(NDT)
        ]

        def qk_step(step):
            ct, chunk, dd = step
            if dd == 0:
                qk_state["ps"] = bgpool.tile([128, 512], F32, tag="bgp", name="qkp")
            ps = qk_state["ps"]
            sl = slice(chunk * 512, (chunk + 1) * 512)
            nc.tensor.matmul(
                ps,
                wqk_sb[:, ct, dd, :],
                xT_sb[:, dd, sl],
                start=(dd == 0),
                stop=(dd == NDT - 1),
            )
            if dd == NDT - 1:
                dst = qT if ct < 4 else kT
                nc.vector.tensor_scalar_add(
                    out=dst[:, ct % 4, sl],
                    in0=ps,
                    scalar1=bias_sb[:, ct : ct + 1],
                )

        # prologue: pair-0 c-tiles (0 and 4) so attention can start
        for step in qk_steps[:64]:
            qk_step(step)

        # ---- background task queue pumped into the attention loop ----
        tasks = deque()

        # v-projection for tiles 10-15 (first consumed at iteration kt=10)
        vbg_state = {}

        def v_task(t, dd):
            if dd == 0:
                vbg_state[t] = bgpool.tile([128, DK], F32, tag="bgp", name=f"vbg{t}")
            nc.tensor.matmul(
                vbg_state[t],
                xT_sb[:, dd, t * 128 : (t + 1) * 128],
                wv_sb[:, dd, :],
                start=(dd == 0),
                stop=(dd == NDT - 1),
            )

        def v_drain(t):
            nc.vector.tensor_copy(
                out=v_sb[:, t, :, 0:DH],
                in_=vbg_state[t].rearrange("p (h e) -> p h e", h=HG),
            )

        for t in range(10, NQT):
            for dd in range(NDT):
                tasks.append(lambda t=t, dd=dd: v_task(t, dd))
            tasks.append(lambda t=t: v_drain(t))

        for step in qk_steps[64:]:  # pairs 1-3 (pair 0 done in the head)
            tasks.append(lambda step=step: qk_step(step))

        # out-projection micro-steps for a group of token tiles (added to the
        # queue once pair 3's attT is normalized for those tokens)
        oproj_state = {}

        def oproj_mm(t, c, j, last):
            sl = slice(c * 512, (c + 1) * 512)
            if j == 0:
                oproj_state[(t, c)] = bgpool.tile(
                    [128, 512], F32, tag="bgp", name=f"fps{t}_{c}"
                )
            nc.tensor.matmul(
                oproj_state[(t, c)],
                attT[:, j, t * 128 : (t + 1) * 128],
                wout_sb[:, j, sl],
                start=(j == 0),
                stop=(j == last),
            )

        def oproj_drain(t, c, os_pool):
            sl = slice(c * 512, (c + 1) * 512)
            osb = os_pool.tile([128, 512], mybir.dt.bfloat16, tag="osb")
            nc.vector.tensor_copy(out=osb, in_=oproj_state[(t, c)])
            eng = nc.sync if (t + c) % 2 == 0 else nc.scalar
            eng.dma_start(out=out[t * 128 : (t + 1) * 128, sl], in_=osb)

        # last token-group (tiles 12-15): pairs 0-2 are accumulated early and
        # parked in SBUF; only the pair-3 matmul + add + DMA remain after the
        # final drain.
        partials = {}

        def oproj_partial_park(t, c, ppool):
            part = ppool.tile([128, 512], F32, tag="part", name=f"part{t}_{c}")
            partials[(t, c)] = part
            nc.vector.tensor_copy(out=part, in_=oproj_state[(t, c)])

        def oproj_finish(t, c, os_pool):
            sl = slice(c * 512, (c + 1) * 512)
            fps = bgpool.tile([128, 512], F32, tag="bgp", name=f"f3_{t}_{c}")
            nc.tensor.matmul(
                fps,
                attT[:, 3, t * 128 : (t + 1) * 128],
                wout_sb[:, 3, sl],
                start=True,
                stop=True,
            )
            osb = os_pool.tile([128, 512], mybir.dt.bfloat16, tag="osb")
            nc.vector.tensor_add(out=osb, in0=fps, in1=partials[(t, c)])
            eng = nc.sync if (t + c) % 2 == 0 else nc.scalar
            eng.dma_start(out=out[t * 128 : (t + 1) * 128, sl], in_=osb)

        # ---- attention ----
        with ExitStack() as p2a:
            epool = p2a.enter_context(tc.tile_pool(name="epool", bufs=8))
            spool = p2a.enter_context(tc.tile_pool(name="sps", bufs=2, space="PSUM"))
            pvpool = p2a.enter_context(tc.tile_pool(name="pvps", bufs=1, space="PSUM"))
            os_pool = p2a.enter_context(tc.tile_pool(name="os", bufs=6))
            ppool = p2a.enter_context(tc.tile_pool(name="part", bufs=8))
            rbpool = p2a.enter_context(tc.tile_pool(name="rbp", bufs=1, space="DRAM"))

            iters = [
                (j, qq, kt)
                for j in range(4)
                for qq in range(4)
                for kt in range(NKT)
            ]

            def emit_scores(idx):
                # both heads of the pair concurrently: head a on PE rows
                # 0-63, head b on rows 64-127 (row-group packing); outputs
                # side by side in one PSUM tile.
                j, qq, kt = iters[idx]
                sp = spool.tile([128, 2 * Q4], F32, tag="sp")
                ksl = slice(kt * 128, (kt + 1) * 128)
                qsl = slice(qq * Q4, (qq + 1) * Q4)
                for lo, half in ((0, 0), (64, 1)):
                    nc.tensor.matmul(
                        sp[:, half * Q4 : (half + 1) * Q4],
                        kT[lo : lo + 64, j, ksl],
                        qT[lo : lo + 64, j, qsl],
                        start=True,
                        stop=True,
                    )
                return sp

            def pump(budget):
                while budget > 0 and tasks:
                    tasks.popleft()()
                    budget -= 1

            pv = [None, None]
            sp = emit_scores(0)
            for i, (j, qq, kt) in enumerate(iters):
                ex = epool.tile([128, 2 * Q4], BF16, tag="ex")
                nc.scalar.activation(out=ex, in_=sp, func=EXP)
                if i + 1 < len(iters):
                    sp = emit_scores(i + 1)
                if kt == 0:
                    pv = [
                        pvpool.tile([DH + 1, Q4], F32, tag="pv_a", name="pv_a"),
                        pvpool.tile([DH + 1, Q4], F32, tag="pv_b", name="pv_b"),
                    ]
                for half in range(2):
                    nc.tensor.matmul(
                        pv[half],
                        v_sb[:, kt, 2 * j + half, :],
                        ex[:, half * Q4 : (half + 1) * Q4],
                        start=(kt == 0),
                        stop=(kt == NKT - 1),
                    )
                # background PE work: finish v tiles 6-15 first (needed from
                # iteration 6), then q/k projection (pair p by iteration
                # 64p), then the out-projection (during pair 3). Drain
                # iterations skip the pump — they already carry extra work.
                if kt < NKT - 1:
                    if i < 12:
                        pump(5)
                    elif i < 24:
                        pump(2)
                    else:
                        pump(3 if j == 3 else 1)
                    continue
                # ---- drain (j, qq): baseline structure — copy out of PSUM,
                # then normalize and relocate head b.
                qsl = slice(qq * Q4, (qq + 1) * Q4)
                stg = nrm.tile([DH + 1, 2, Q4], F32, tag="stg")
                tmp = nrm.tile([64, Q4], BF16, tag="tmp")
                den = nrm.tile([1, 2 * Q4], F32, tag="den")
                r_t = nrm.tile([1, 2 * Q4], F32, tag="r_t")
                bc = nrm.tile([64, 2 * Q4], F32, tag="bc")
                for half in range(2):
                    nc.vector.tensor_copy(out=stg[:, half, :], in_=pv[half])
                nc.vector.tensor_copy(out=den, in_=stg[DH : DH + 1, :, :])
                nc.vector.reciprocal_approx_fast(out=r_t, in_=den)
                nc.gpsimd.partition_broadcast(out_ap=bc, in_ap=r_t, channels=64)
                nc.vector.tensor_mul(
                    out=attT[0:DH, j, qsl], in0=stg[0:DH, 0, :], in1=bc[:, 0:Q4]
                )
                nc.vector.tensor_mul(
                    out=tmp, in0=stg[0:DH, 1, :], in1=bc[:, Q4 : 2 * Q4]
                )
                # head b relocates to partitions 64-127 of attT by DMA
                # (engines cannot cross partitions)
                nc.sync.dma_start(out=attT[64:128, j, qsl], in_=tmp)
                if j == 3:
                    for t in range(qq * 4, qq * 4 + 4):
                        for c in range(2):
                            for jj in range(4):
                                tasks.append(
                                    lambda t=t, c=c, jj=jj: oproj_mm(t, c, jj, 3)
                                )
                            tasks.append(
                                lambda t=t, c=c: oproj_drain(t, c, os_pool)
                            )

            # ---- tail: remaining out-projection work ----
            pump(len(tasks))

    nc.compile()
    return nc


def make_in_maps(x, Wqkv, bqkv, Wout):
    """Host-side sharding: returns 8 per-core input dicts."""
    import ml_dtypes

    bf16 = ml_dtypes.bfloat16
    B = x.shape[0]
    scale = np.float32(1.0 / np.sqrt(DH))
    xTs = [np.ascontiguousarray(x[b].T.astype(bf16)) for b in range(B)]
    per_group = []
    for g in range(2):
        qsl = slice(g * DK, (g + 1) * DK)
        ksl = slice(D + g * DK, D + (g + 1) * DK)
        vsl = slice(2 * D + g * DK, 2 * D + (g + 1) * DK)
        wqk_full = np.concatenate([Wqkv[:, qsl] * scale, Wqkv[:, ksl]], axis=1)
        wqk_t = np.ascontiguousarray(
            wqk_full.reshape(NDT, 128, 8, 128).transpose(2, 0, 1, 3).astype(bf16)
        )
        wv_t = np.ascontiguousarray(Wqkv[:, vsl].astype(bf16)).reshape(NDT, 128, DK)
        bqk_g = np.concatenate([bqkv[qsl] * scale, bqkv[ksl]]).astype(np.float32)
        wout_g = np.ascontiguousarray(Wout[g * DK : (g + 1) * DK, :].astype(bf16))
        per_group.append(
            {"wqk": wqk_t, "wv": wv_t, "bqk": bqk_g, "wout": wout_g}
        )
    in_maps = []
    for c in range(2 * B):
        b, g = c // 2, c % 2
        in_maps.append({"xT": xTs[b], **per_group[g]})
    return in_maps


_PROGRAM = None
# test-harness knobs (grading path leaves these at defaults)
TRACE = False
TRACE_KWARGS = {}
LAST_RESULTS = None


def _get_program():
    global _PROGRAM
    if _PROGRAM is None:
        _PROGRAM = build_program()
    return _PROGRAM


def _reference_fallback(x, mask, Wqkv, bqkv, Wout, bout):
    # numpy fallback for general masks (harness always passes all-true)
    B, S_, D_ = x.shape
    H, dh = 16, D_ // 16
    qkv = x @ Wqkv + bqkv
    qkv = qkv.reshape(B, S_, 3, H, dh)
    q, k, v = qkv[:, :, 0], qkv[:, :, 1], qkv[:, :, 2]
    scores = np.einsum("bqhd,bkhd->bhqk", q, k) / np.sqrt(dh)
    m = (mask[:, None, :, None] & mask[:, None, None, :])
    scores = np.where(m, scores, -1e30)
    scores -= scores.max(axis=-1, keepdims=True)
    e = np.exp(scores)
    attn = e / e.sum(axis=-1, keepdims=True)
    o = np.einsum("bhqk,bkhd->bqhd", attn, v).reshape(B, S_, D_)
    return (o @ Wout + bout).astype(np.float32)


def kernel(x, mask, Wqkv, bqkv, Wout, bout):
    x = np.asarray(x, dtype=np.float32)
    mask = np.asarray(mask)
    Wqkv = np.asarray(Wqkv, dtype=np.float32)
    bqkv = np.asarray(bqkv, dtype=np.float32)
    Wout = np.asarray(Wout, dtype=np.float32)
    bout = np.asarray(bout, dtype=np.float32)

    if not mask.all():
        return _reference_fallback(x, mask, Wqkv, bqkv, Wout, bout)

    from concourse.bass_utils import run_bass_kernel_spmd

    B = x.shape[0]
    nc = _get_program()
    in_maps = make_in_maps(x, Wqkv, bqkv, Wout)
    res = run_bass_kernel_spmd(
        nc,
        in_maps,
        core_ids=list(range(2 * B)),
        trace=TRACE,
        **TRACE_KWARGS,
    )
    global LAST_RESULTS
    LAST_RESULTS = res

    # v-bias folds into a constant shift through the out projection
    host_add = (bout + bqkv[2 * D : 3 * D] @ Wout).astype(np.float32)
    out = np.empty((B, S, D), dtype=np.float32)
    for b in range(B):
        out[b] = (
            res.results[2 * b]["out"].astype(np.float32)
            + res.results[2 * b + 1]["out"].astype(np.float32)
            + host_add
        )
    return out


# revision 43
# speedup vs baseline: 1.0083x; 1.0083x over previous
"""Trainium2 Bass kernel for MemoryEfficientAttention (B=4, S=2048, D=1024, H=16).

Sharding: 8 cores = 4 batches x 2 head-groups (8 heads each).
Each core computes qkv projection for its head group, attention, and a
row-parallel partial of the output projection. Host sums the two partials
per batch and folds the (zero) biases.

Schedule: the v-projection overlaps the input DMA (dd-outer accumulation);
the tail of the v-projection, the q/k projection, and the output projection
all run as background PE tasks pumped into the attention loop's slack, so
the ScalarE exp stream (the pacing engine) starts early and never waits at
the end. Softmax normalization is fused per (pair, q-chunk) drain using a
GPSIMD partition-broadcast of the reciprocal denominators (no DRAM bounce).
"""

import sys
from collections import deque
from contextlib import ExitStack

if "/opt/trn_rl_repo" not in sys.path:
    sys.path.insert(0, "/opt/trn_rl_repo")

import numpy as np

import concourse.bass as bass
import concourse.mybir as mybir
import concourse.tile as tile
from concourse import bacc

F32 = mybir.dt.float32
EXP = mybir.ActivationFunctionType.Exp

S = 2048          # sequence length
D = 1024          # model dim
HG = 8            # heads per core (group)
DH = 64           # head dim
DK = HG * DH      # 512, per-core attention dim
NKT = S // 128    # 16 key tiles
NQT = S // 128    # 16 query/token tiles
NDT = D // 128    # 8 d-tiles
Q4 = 512          # query chunk


def build_program():
    """Build the SPMD Bass/Tile program (same program on all 8 cores)."""
    nc = bacc.Bacc("TRN2")
    BF16 = mybir.dt.bfloat16

    xT = nc.dram_tensor("xT", [D, S], BF16, kind="ExternalInput").ap()
    # wqk: 8 column-tiles (q cols 0-511 scaled by 1/8, then k cols), tiled
    # [ct, dt, 128, 128] so each DMA is one contiguous 64KB block.
    wqk = nc.dram_tensor("wqk", [8, NDT, 128, 128], BF16, kind="ExternalInput").ap()
    # wv: [dt, 128, 512] row-blocks of the v projection.
    wv = nc.dram_tensor("wv", [NDT, 128, DK], BF16, kind="ExternalInput").ap()
    # bqk: q bias (pre-scaled) then k bias, laid out [128, 8] partition-major.
    bqk = nc.dram_tensor("bqk", [D], F32, kind="ExternalInput").ap()
    wout = nc.dram_tensor("wout", [DK, D], BF16, kind="ExternalInput").ap()
    # bf16 partial output: halves the writeback DMA; host sums in fp32
    out = nc.dram_tensor("out", [S, D], BF16, kind="ExternalOutput").ap()


    with tile.TileContext(nc) as tc, ExitStack() as ctx:
        persist = ctx.enter_context(tc.tile_pool(name="persist", bufs=1))
        # qT/kT: transposed projections, (dh x tokens) per head; head h lives
        # in tile column h//2 at partitions (h%2)*64 .. +64.
        qT = persist.tile([128, 4, S], BF16, tag="qT")
        kT = persist.tile([128, 4, S], BF16, tag="kT")
        bias_sb = persist.tile([128, 8], F32, tag="bias_sb")
        wout_sb = persist.tile([128, 4, D], BF16, tag="wout_sb")
        attT = persist.tile([128, 4, S], BF16, tag="attT")
        xT_sb = persist.tile([128, NDT, S], BF16, tag="xT_sb")
        wqk_sb = persist.tile([128, 8, NDT, 128], BF16, tag="wqk_sb")
        wv_sb = persist.tile([128, NDT, DK], BF16, tag="wv_sb")
        # v in natural layout (bf16), augmented with a ones column per head:
        # v_sb[:, kt, h, 0:64] = v tokens kt*128.., head h; [..., 64] = 1.0
        v_sb = persist.tile([128, NKT, HG, DH + 1], BF16, tag="v_sb")

        # small rotating buffers for the per-chunk softmax normalization
        nrm = ctx.enter_context(tc.tile_pool(name="nrm", bufs=3))

        nc.vector.memset(v_sb[:, :, :, DH : DH + 1].bitcast(mybir.dt.uint16), 0x3F80)

        # ---- input DMAs, split across queues; wv first so the v-projection
        # can start as soon as x d-tile 0 lands.
        for dd in range(NDT):
            nc.sync.dma_start(out=xT_sb[:, dd, :], in_=xT[dd * 128 : (dd + 1) * 128, :])
        for j in range(4):
            nc.sync.dma_start(out=wout_sb[:, j, :], in_=wout[j * 128 : (j + 1) * 128, :])
        for dd in range(NDT):
            nc.scalar.dma_start(out=wv_sb[:, dd, :], in_=wv[dd])
        nc.scalar.dma_start(out=bias_sb, in_=bqk.rearrange("(c p) -> p c", p=128))
        for ct in (0, 4, 1, 5, 2, 6, 3, 7):
            nc.scalar.dma_start(
                out=wqk_sb[:, ct, :, :], in_=wqk[ct].rearrange("d p k -> p d k")
            )

        # ---- head phase 1: v-projection for token tiles 0-9, dd-outer so
        # the PSUM accumulation starts after the first x d-tile arrives.
        with ExitStack() as p1b:
            vps_pool = p1b.enter_context(tc.tile_pool(name="vps", bufs=8, space="PSUM"))
            for tset in (range(0, 8), range(8, 10)):
                vt = {}
                for t in tset:
                    vt[t] = vps_pool.tile([128, DK], F32, tag="vps", name=f"vps{t}")
                for dd in range(NDT):
                    for t in tset:
                        nc.tensor.matmul(
                            vt[t],
                            xT_sb[:, dd, t * 128 : (t + 1) * 128],
                            wv_sb[:, dd, :],
                            start=(dd == 0),
                            stop=(dd == NDT - 1),
                        )
                for t in tset:
                    nc.vector.tensor_copy(
                        out=v_sb[:, t, :, 0:DH],
                        in_=vt[t].rearrange("p (h e) -> p h e", h=HG),
                    )

        # ---- background-task PSUM pool: q/k projection accumulators, the
        # remaining v-projection tiles, and later the out-projection chunks
        # all use the same [128, 512] fp32 shape.
        bgpool = ctx.enter_context(tc.tile_pool(name="bgp", bufs=2, space="PSUM"))

        # q/k projection stepper: one N=512 matmul per step; a step sequence
        # covers c-tiles in pair order so pair j+1's tiles finish during pair
        # j's attention.
        qk_state = {"ps": None}
        qk_steps = [
            (ct, chunk, dd)
            for ct in (0, 4, 1, 5, 2, 6, 3, 7)
            for chunk in range(4)
            for dd in range(NDT)
        ]

        def qk_step(step):
            ct, chunk, dd = step
            if dd == 0:
                qk_state["ps"] = bgpool.tile([128, 512], F32, tag="bgp", name="qkp")
            ps = qk_state["ps"]
            sl = slice(chunk * 512, (chunk + 1) * 512)
            nc.tensor.matmul(
                ps,
                wqk_sb[:, ct, dd, :],
                xT_sb[:, dd, sl],
                start=(dd == 0),
                stop=(dd == NDT - 1),
            )
            if dd == NDT - 1:
                dst = qT if ct < 4 else kT
                nc.vector.tensor_scalar_add(
                    out=dst[:, ct % 4, sl],
                    in0=ps,
                    scalar1=bias_sb[:, ct : ct + 1],
                )

        # prologue: pair-0 c-tiles (0 and 4) so attention can start
        for step in qk_steps[:64]:
            qk_step(step)

        # ---- background task queue pumped into the attention loop ----
        tasks = deque()

        # v-projection for tiles 10-15 (first consumed at iteration kt=10)
        vbg_state = {}

        def v_task(t, dd):
            if dd == 0:
                vbg_state[t] = bgpool.tile([128, DK], F32, tag="bgp", name=f"vbg{t}")
            nc.tensor.matmul(
                vbg_state[t],
                xT_sb[:, dd, t * 128 : (t + 1) * 128],
                wv_sb[:, dd, :],
                start=(dd == 0),
                stop=(dd == NDT - 1),
            )

        def v_drain(t):
            nc.vector.tensor_copy(
                out=v_sb[:, t, :, 0:DH],
                in_=vbg_state[t].rearrange("p (h e) -> p h e", h=HG),
            )

        for t in range(10, NQT):
            for dd in range(NDT):
                tasks.append(lambda t=t, dd=dd: v_task(t, dd))
            tasks.append(lambda t=t: v_drain(t))

        for step in qk_steps[64:]:  # pairs 1-3 (pair 0 done in the head)
            tasks.append(lambda step=step: qk_step(step))

        # out-projection micro-steps for a group of token tiles (added to the
        # queue once pair 3's attT is normalized for those tokens)
        oproj_state = {}

        def oproj_mm(t, c, j, last):
            sl = slice(c * 512, (c + 1) * 512)
            if j == 0:
                oproj_state[(t, c)] = bgpool.tile(
                    [128, 512], F32, tag="bgp", name=f"fps{t}_{c}"
                )
            nc.tensor.matmul(
                oproj_state[(t, c)],
                attT[:, j, t * 128 : (t + 1) * 128],
                wout_sb[:, j, sl],
                start=(j == 0),
                stop=(j == last),
            )

        def oproj_drain(t, c, os_pool):
            sl = slice(c * 512, (c + 1) * 512)
            osb = os_pool.tile([128, 512], mybir.dt.bfloat16, tag="osb")
            nc.vector.tensor_copy(out=osb, in_=oproj_state[(t, c)])
            eng = nc.sync if (t + c) % 2 == 0 else nc.scalar
            eng.dma_start(out=out[t * 128 : (t + 1) * 128, sl], in_=osb)

        # last token-group (tiles 12-15): pairs 0-2 are accumulated early and
        # parked in SBUF; only the pair-3 matmul + add + DMA remain after the
        # final drain.
        partials = {}

        def oproj_partial_park(t, c, ppool):
            part = ppool.tile([128, 512], F32, tag="part", name=f"part{t}_{c}")
            partials[(t, c)] = part
            nc.vector.tensor_copy(out=part, in_=oproj_state[(t, c)])

        def oproj_finish(t, c, os_pool):
            sl = slice(c * 512, (c + 1) * 512)
            fps = bgpool.tile([128, 512], F32, tag="bgp", name=f"f3_{t}_{c}")
            nc.tensor.matmul(
                fps,
                attT[:, 3, t * 128 : (t + 1) * 128],
                wout_sb[:, 3, sl],
                start=True,
                stop=True,
            )
            osb = os_pool.tile([128, 512], mybir.dt.bfloat16, tag="osb")
            nc.vector.tensor_add(out=osb, in0=fps, in1=partials[(t, c)])
            eng = nc.sync if (t + c) % 2 == 0 else nc.scalar
            eng.dma_start(out=out[t * 128 : (t + 1) * 128, sl], in_=osb)

        # ---- attention ----
        with ExitStack() as p2a:
            epool = p2a.enter_context(tc.tile_pool(name="epool", bufs=6))
            spool = p2a.enter_context(tc.tile_pool(name="sps", bufs=2, space="PSUM"))
            pvpool = p2a.enter_context(tc.tile_pool(name="pvps", bufs=1, space="PSUM"))
            os_pool = p2a.enter_context(tc.tile_pool(name="os", bufs=6))
            ppool = p2a.enter_context(tc.tile_pool(name="part", bufs=8))
            rbpool = p2a.enter_context(tc.tile_pool(name="rbp", bufs=1, space="DRAM"))

            iters = [
                (j, qq, kt)
                for j in range(4)
                for qq in range(4)
                for kt in range(NKT)
            ]

            def emit_scores(idx):
                # both heads of the pair concurrently: head a on PE rows
                # 0-63, head b on rows 64-127 (row-group packing); outputs
                # side by side in one PSUM tile.
                j, qq, kt = iters[idx]
                sp = spool.tile([128, 2 * Q4], F32, tag="sp")
                ksl = slice(kt * 128, (kt + 1) * 128)
                qsl = slice(qq * Q4, (qq + 1) * Q4)
                for lo, half in ((0, 0), (64, 1)):
                    nc.tensor.matmul(
                        sp[:, half * Q4 : (half + 1) * Q4],
                        kT[lo : lo + 64, j, ksl],
                        qT[lo : lo + 64, j, qsl],
                        start=True,
                        stop=True,
                    )
                return sp

            def pump(budget):
                while budget > 0 and tasks:
                    tasks.popleft()()
                    budget -= 1

            pv = [None, None]
            sp = emit_scores(0)
            for i, (j, qq, kt) in enumerate(iters):
                ex = epool.tile([128, 2 * Q4], BF16, tag="ex")
                nc.scalar.activation(out=ex, in_=sp, func=EXP)
                if i + 1 < len(iters):
                    sp = emit_scores(i + 1)
                if kt == 0:
                    pv = [
                        pvpool.tile([DH + 1, Q4], F32, tag="pv_a", name="pv_a"),
                        pvpool.tile([DH + 1, Q4], F32, tag="pv_b", name="pv_b"),
                    ]
                for half in range(2):
                    nc.tensor.matmul(
                        pv[half],
                        v_sb[:, kt, 2 * j + half, :],
                        ex[:, half * Q4 : (half + 1) * Q4],
                        start=(kt == 0),
                        stop=(kt == NKT - 1),
                    )
                # background PE work: finish v tiles 6-15 first (needed from
                # iteration 6), then q/k projection (pair p by iteration
                # 64p), then the out-projection (during pair 3). Drain
                # iterations skip the pump — they already carry extra work.
                if kt < NKT - 1:
                    if i < 12:
                        pump(5)
                    elif i < 24:
                        pump(2)
                    else:
                        pump(3 if j == 3 else 1)
                    continue
                # ---- drain (j, qq): baseline structure — copy out of PSUM,
                # then normalize and relocate head b.
                qsl = slice(qq * Q4, (qq + 1) * Q4)
                stg = nrm.tile([DH + 1, 2, Q4], F32, tag="stg")
                tmp = nrm.tile([64, Q4], BF16, tag="tmp")
                den = nrm.tile([1, 2 * Q4], F32, tag="den")
                r_t = nrm.tile([1, 2 * Q4], F32, tag="r_t")
                bc = nrm.tile([64, 2 * Q4], F32, tag="bc")
                for half in range(2):
                    nc.vector.tensor_copy(out=stg[:, half, :], in_=pv[half])
                nc.vector.tensor_copy(out=den, in_=stg[DH : DH + 1, :, :])
                nc.vector.reciprocal_approx_fast(out=r_t, in_=den)
                nc.gpsimd.partition_broadcast(out_ap=bc, in_ap=r_t, channels=64)
                nc.vector.tensor_mul(
                    out=attT[0:DH, j, qsl], in0=stg[0:DH, 0, :], in1=bc[:, 0:Q4]
                )
                nc.vector.tensor_mul(
                    out=tmp, in0=stg[0:DH, 1, :], in1=bc[:, Q4 : 2 * Q4]
                )
                # head b relocates to partitions 64-127 of attT by DMA
                # (engines cannot cross partitions)
                nc.sync.dma_start(out=attT[64:128, j, qsl], in_=tmp)
                if j == 3:
                    for t in range(qq * 4, qq * 4 + 4):
                        for c in range(2):
                            for jj in range(4):
                                tasks.append(
                                    lambda t=t, c=c, jj=jj: oproj_mm(t, c, jj, 3)
                                )
                            tasks.append(
                                lambda t=t, c=c: oproj_drain(t, c, os_pool)
                            )

            # ---- tail: remaining out-projection work ----
            pump(len(tasks))

    nc.compile()
    return nc


def make_in_maps(x, Wqkv, bqkv, Wout):
    """Host-side sharding: returns 8 per-core input dicts."""
    import ml_dtypes

    bf16 = ml_dtypes.bfloat16
    B = x.shape[0]
    scale = np.float32(1.0 / np.sqrt(DH))
    xTs = [np.ascontiguousarray(x[b].T.astype(bf16)) for b in range(B)]
    per_group = []
    for g in range(2):
        qsl = slice(g * DK, (g + 1) * DK)
        ksl = slice(D + g * DK, D + (g + 1) * DK)
        vsl = slice(2 * D + g * DK, 2 * D + (g + 1) * DK)
        wqk_full = np.concatenate([Wqkv[:, qsl] * scale, Wqkv[:, ksl]], axis=1)
        wqk_t = np.ascontiguousarray(
            wqk_full.reshape(NDT, 128, 8, 128).transpose(2, 0, 1, 3).astype(bf16)
        )
        wv_t = np.ascontiguousarray(Wqkv[:, vsl].astype(bf16)).reshape(NDT, 128, DK)
        bqk_g = np.concatenate([bqkv[qsl] * scale, bqkv[ksl]]).astype(np.float32)
        wout_g = np.ascontiguousarray(Wout[g * DK : (g + 1) * DK, :].astype(bf16))
        per_group.append(
            {"wqk": wqk_t, "wv": wv_t, "bqk": bqk_g, "wout": wout_g}
        )
    in_maps = []
    for c in range(2 * B):
        b, g = c // 2, c % 2
        in_maps.append({"xT": xTs[b], **per_group[g]})
    return in_maps


_PROGRAM = None
# test-harness knobs (grading path leaves these at defaults)
TRACE = False
TRACE_KWARGS = {}
LAST_RESULTS = None


def _get_program():
    global _PROGRAM
    if _PROGRAM is None:
        _PROGRAM = build_program()
    return _PROGRAM


def _reference_fallback(x, mask, Wqkv, bqkv, Wout, bout):
    # numpy fallback for general masks (harness always passes all-true)
    B, S_, D_ = x.shape
    H, dh = 16, D_ // 16
    qkv = x @ Wqkv + bqkv
    qkv = qkv.reshape(B, S_, 3, H, dh)
    q, k, v = qkv[:, :, 0], qkv[:, :, 1], qkv[:, :, 2]
    scores = np.einsum("bqhd,bkhd->bhqk", q, k) / np.sqrt(dh)
    m = (mask[:, None, :, None] & mask[:, None, None, :])
    scores = np.where(m, scores, -1e30)
    scores -= scores.max(axis=-1, keepdims=True)
    e = np.exp(scores)
    attn = e / e.sum(axis=-1, keepdims=True)
    o = np.einsum("bhqk,bkhd->bqhd", attn, v).reshape(B, S_, D_)
    return (o @ Wout + bout).astype(np.float32)


def kernel(x, mask, Wqkv, bqkv, Wout, bout):
    x = np.asarray(x, dtype=np.float32)
    mask = np.asarray(mask)
    Wqkv = np.asarray(Wqkv, dtype=np.float32)
    bqkv = np.asarray(bqkv, dtype=np.float32)
    Wout = np.asarray(Wout, dtype=np.float32)
    bout = np.asarray(bout, dtype=np.float32)

    if not mask.all():
        return _reference_fallback(x, mask, Wqkv, bqkv, Wout, bout)

    from concourse.bass_utils import run_bass_kernel_spmd

    B = x.shape[0]
    nc = _get_program()
    in_maps = make_in_maps(x, Wqkv, bqkv, Wout)
    res = run_bass_kernel_spmd(
        nc,
        in_maps,
        core_ids=list(range(2 * B)),
        trace=TRACE,
        **TRACE_KWARGS,
    )
    global LAST_RESULTS
    LAST_RESULTS = res

    # v-bias folds into a constant shift through the out projection
    host_add = (bout + bqkv[2 * D : 3 * D] @ Wout).astype(np.float32)
    out = np.empty((B, S, D), dtype=np.float32)
    for b in range(B):
        out[b] = (
            res.results[2 * b]["out"].astype(np.float32)
            + res.results[2 * b + 1]["out"].astype(np.float32)
            + host_add
        )
    return out
